# revision 1
# baseline (speedup 1.0000x reference)
"""CfC (closed-form continuous-time) cell kernel for Trainium2, 8 NeuronCores.

Reference computation (B=8192, IN=256, H=512, all fp32):
    g     = sigmoid(x @ W_gx.T + b_gx + h @ W_gh.T + gate_b)        [B, H]
    f     = tanh(cat([x, h]) @ W_backbone.T + b_backbone)           [B, H]
    tau   = softplus(log_tau) + |g|          (g in (0,1) so |g| == g)
    decay = exp(-delta_t[:, None] * tau)
    out   = decay * h + (1 - decay) * f

Strategy: data-parallel over B (1024 rows per core), weights replicated.
Device work happens in "feature-major" (transposed) layout: activations ship
as xh^T [768, B_shard] so the contraction dim lands on SBUF partitions with
no on-device transposes.  Gate and backbone share the same moving operand
(xh^T); their transposed weights are stacked into one [768, 1024] tensor.
Matmul inputs are fp16 (full PE rate + fast weight load; ~4x less rounding
than bf16 and far from fp16 range limits).  The elementwise chain also runs
in fp16 — the DVE runs 16-bit tensor_tensor at 2x — with the final combine
written out in fp32.  Per-feature vectors (biases, softplus(log_tau)) are
per-partition scalars fused into ACT bias/scale slots; -delta_t arrives
pre-broadcast from the host (a step-0 broadcast DMA degenerates to 4-byte
descriptors).  sigmoid(u) = 0.5 + 0.5*tanh(u/2) because Sigmoid and Exp never
share an ACT table; softplus(x) = ln(1+exp(x)) likewise.  All streamed
tensors are packed partition-major on the host so every DMA descriptor row is
>= 4KB.  The packed [128, 4*B_shard] per-core results are unpacked, gathered
and transposed on the host.
"""

from contextlib import ExitStack

import numpy as np

import concourse.bass as bass
import concourse.mybir as mybir
import concourse.tile as tile
from concourse import bacc
from concourse.bass_utils import run_bass_kernel_spmd

B, IN, H = 8192, 256, 512
NCORES = 8
BS = B // NCORES          # 1024 batch rows per core
KIN = IN + H              # 768 contraction dim
KT = KIN // 128           # 6 k-tiles
NJ = H // 128             # 4 partition tiles per output matrix
NCHUNK = 512              # matmul moving free dim per PSUM bank
NCH = BS // NCHUNK        # 2 b-chunks per core

F32 = mybir.dt.float32
FP16 = mybir.dt.float16
AF = mybir.ActivationFunctionType
OP = mybir.AluOpType

TRACE = False             # test.py flips this for profiled runs
LAST_RESULT = None        # BassKernelResults of the most recent run

_NC_CACHE = None


def _body(tc, xhT, WT, consts, negdt, outP):
    nc = tc.nc
    with ExitStack() as ctx:
        singles = ctx.enter_context(tc.tile_pool(name="singles", bufs=1))
        work = ctx.enter_context(tc.tile_pool(name="work", bufs=3))
        psg = ctx.enter_context(tc.tile_pool(name="psg", bufs=2, space="PSUM"))
        psf = ctx.enter_context(tc.tile_pool(name="psf", bufs=2, space="PSUM"))

        # Persistent SBUF tensors.  xhT/WT arrive partition-major-packed:
        # [128, KT*cols] with each partition row contiguous in DRAM.
        xh_sb = singles.tile([128, KT, BS], FP16, tag="xh")
        w_sb = singles.tile([128, KT, 2 * H], FP16, tag="w")
        cst = singles.tile([128, 3, NJ], F32, tag="cst")
        stau = singles.tile([128, NJ], F32, tag="stau")
        ndt = singles.tile([128, BS], F32, tag="ndt")

        xh_v = xhT.rearrange("p (g c) -> p g c", g=KT // 2)   # k-pair panels
        w_v = WT.rearrange("p (g c) -> p g c", g=KT // 2)

        # Stream inputs in matmul consumption order, most-urgent first: the
        # k-pair-0 panels gate the first matmuls.  Each DMA moves 512KB with
        # 4KB rows.
        for g in range(KT // 2):
            nc.sync.dma_start(
                out=xh_sb[:, 2 * g:2 * g + 2, :].rearrange("p k b -> p (k b)"),
                in_=xh_v[:, g, :],
            )
            nc.sync.dma_start(
                out=w_sb[:, 2 * g:2 * g + 2, :].rearrange("p k n -> p (k n)"),
                in_=w_v[:, g, :],
            )
        # consts is [bg/2 | bb | log_tau] each [H]; column j of the SBUF tile
        # is that vector's j-th 128-row slice (per-partition scalars).
        nc.gpsimd.dma_start(
            out=cst, in_=consts.rearrange("(c j p) -> p c j", p=128, j=NJ)
        )
        # -delta_t pre-broadcast on the host (fp16, 2KB rows, one-time 256KB)
        nc.gpsimd.dma_start(out=ndt, in_=negdt)

        # softplus(log_tau) = ln(1 + exp(log_tau)); the ACT table with
        # Softplus itself never ships with Exp/Tanh, so build it from Ln/Exp.
        # stau holds softplus(log_tau) + 0.5 — the 0.5 is the constant term of
        # sigmoid(u) = 0.5 + 0.5*tanh(u/2), folded into tau below.
        e0 = singles.tile([128, NJ], F32, tag="e0")
        nc.scalar.activation(out=e0, in_=cst[:, 2, :], func=AF.Exp)
        nc.vector.tensor_scalar_add(e0, e0, 1.0)
        nc.scalar.activation(out=stau, in_=e0, func=AF.Ln)
        nc.vector.tensor_scalar_add(stau, stau, 0.5)

        # Process output tiles in j-pairs: each j gets a [128, BS] 2-bank PSUM
        # accumulator (both b-chunks side by side), so the elementwise chain
        # runs on [128, BS] units — half the per-op overhead — while 2 pairs
        # x (gate+backbone) x 2 banks fill all 8 PSUM banks.
        for jh in range(NJ // 2):
            jpair = [2 * jh, 2 * jh + 1]
            zg = {j: psg.tile([128, BS], F32, tag="zg", name=f"zg_{j}") for j in jpair}
            zf = {j: psf.tile([128, BS], F32, tag="zf", name=f"zf_{j}") for j in jpair}
            for n in range(NCH):
                bsl = slice(n * NCHUNK, (n + 1) * NCHUNK)
                for k in range(KT):
                    rhs = xh_sb[:, k, bsl]
                    for j in jpair:
                        nc.tensor.matmul(
                            zg[j][:, bsl],
                            w_sb[:, k, j * 128:(j + 1) * 128],
                            rhs,
                            start=(k == 0),
                            stop=(k == KT - 1),
                        )
                    for j in jpair:
                        nc.tensor.matmul(
                            zf[j][:, bsl],
                            w_sb[:, k, H + j * 128:H + (j + 1) * 128],
                            rhs,
                            start=(k == 0),
                            stop=(k == KT - 1),
                        )
            for j in jpair:
                tg = work.tile([128, BS], FP16, tag="tg", name=f"tg_{j}")
                f = work.tile([128, BS], FP16, tag="f", name=f"f_{j}")
                tau = work.tile([128, BS], F32, tag="tau", name=f"tau_{j}")
                t = work.tile([128, BS], F32, tag="t", name=f"t_{j}")
                decay = work.tile([128, BS], F32, tag="decay", name=f"decay_{j}")
                hmf = work.tile([128, BS], F32, tag="hmf", name=f"hmf_{j}")
                p = work.tile([128, BS], F32, tag="p", name=f"p_{j}")

                # sigmoid(zg + bg) = 0.5 + 0.5*tanh((zg + bg)/2): Sigmoid never
                # shares an ACT table with Exp, but Tanh does.  cst slot 0
                # holds bg/2; the 0.5 offsets are folded into stau.
                nc.scalar.activation(
                    out=tg, in_=zg[j], func=AF.Tanh, bias=cst[:, 0, j:j + 1],
                    scale=0.5,
                )
                nc.scalar.activation(
                    out=f, in_=zf[j], func=AF.Tanh, bias=cst[:, 1, j:j + 1]
                )
                # tau = g + softplus(log_tau) = 0.5*tg + (softplus+0.5)
                nc.vector.tensor_scalar(
                    out=tau, in0=tg, scalar1=0.5, scalar2=stau[:, j:j + 1],
                    op0=OP.mult, op1=OP.add,
                )
                # t = -dt * tau
                nc.vector.tensor_mul(out=t, in0=tau, in1=ndt)
                nc.scalar.activation(out=decay, in_=t, func=AF.Exp)
                # out = f + decay * (h - f); h rows live in xh_sb k-tiles 2..5
                nc.vector.tensor_sub(out=hmf, in0=xh_sb[:, 2 + j, :], in1=f)
                nc.vector.tensor_mul(out=p, in0=decay, in1=hmf)
                # Final combine in fp32, split by b-chunk so the two output
                # DMAs land on separate queues.
                for n in range(NCH):
                    bsl = slice(n * NCHUNK, (n + 1) * NCHUNK)
                    o = work.tile([128, NCHUNK], F32, tag="o", name=f"o_{j}_{n}")
                    nc.vector.tensor_add(out=o, in0=p[:, bsl], in1=f[:, bsl])
                    nc.sync.dma_start(
                        out=outP[:, j * BS + n * NCHUNK:
                                 j * BS + (n + 1) * NCHUNK],
                        in_=o,
                    )


def build_nc():
    nc = bacc.Bacc(
        "TRN2",
        target_bir_lowering=False,
        debug=False,
        enable_asserts=False,
        num_devices=NCORES,
    )
    # Partition-major packed streams: row p holds that partition's entire
    # contiguous payload (>= 4KB per DMA descriptor row).
    xhT = nc.dram_tensor("xhT", [128, KT * BS], FP16, kind="ExternalInput").ap()
    WT = nc.dram_tensor("WT", [128, KT * 2 * H], FP16, kind="ExternalInput").ap()
    consts = nc.dram_tensor("consts", [3 * H], F32, kind="ExternalInput").ap()
    negdt = nc.dram_tensor("negdt", [128, BS], F32, kind="ExternalInput").ap()
    outP = nc.dram_tensor("outP", [128, NJ * BS], F32, kind="ExternalOutput").ap()
    with tile.TileContext(nc) as tc:
        _body(tc, xhT, WT, consts, negdt, outP)
    nc.compile()
    return nc


def _get_nc():
    global _NC_CACHE
    if _NC_CACHE is None:
        _NC_CACHE = build_nc()
    return _NC_CACHE


def _pack_pmajor(a, kt):
    """[kt*128, C] -> [128, kt*C]: partition-major pack so each of the 128
    DMA rows is contiguous in DRAM."""
    c = a.shape[1]
    return np.ascontiguousarray(
        a.reshape(kt, 128, c).transpose(1, 0, 2).reshape(128, kt * c)
    )


def make_in_maps(x, h, delta_t, W_backbone, b_backbone, W_gx, b_gx, W_gh,
                 gate_b, log_tau):
    f32 = np.float32
    xh = np.concatenate(
        [np.asarray(x, f32), np.asarray(h, f32)], axis=1
    )                                                   # [B, 768]
    xhT = np.ascontiguousarray(xh.T).astype(np.float16)  # [768, B]
    WgT = np.concatenate(
        [np.asarray(W_gx, f32), np.asarray(W_gh, f32)], axis=1
    ).T                                                 # [768, H]
    WT = np.concatenate(
        [WgT, np.asarray(W_backbone, f32).T], axis=1
    ).astype(np.float16)                                # [768, 2H]
    WT_p = _pack_pmajor(WT, KT)                         # [128, KT*1024]
    consts = np.concatenate(
        [
            (np.asarray(b_gx, f32) + np.asarray(gate_b, f32)) * 0.5,
            np.asarray(b_backbone, f32),
            np.asarray(log_tau, f32),
        ]
    ).astype(f32)                                       # [3H]
    negdt = (-np.asarray(delta_t, f32)).astype(f32)          # [B]

    in_maps = []
    for c in range(NCORES):
        sl = slice(c * BS, (c + 1) * BS)
        in_maps.append(
            {
                "xhT": _pack_pmajor(xhT[:, sl], KT),
                "WT": WT_p,
                "consts": consts,
                "negdt": np.ascontiguousarray(
                    np.broadcast_to(negdt[sl][None, :], (128, BS))
                ),
            }
        )
    return in_maps


def kernel(x, h, delta_t, W_backbone, b_backbone, W_gx, b_gx, W_gh, gate_b,
           log_tau):
    global LAST_RESULT
    in_maps = make_in_maps(x, h, delta_t, W_backbone, b_backbone, W_gx, b_gx,
                           W_gh, gate_b, log_tau)
    nc = _get_nc()
    res = run_bass_kernel_spmd(
        nc, in_maps, core_ids=list(range(NCORES)), trace=TRACE
    )
    LAST_RESULT = res
    # outP is [128, NJ*BS] partition-major; unpack to [H, BS] then gather.
    outs = []
    for r in res.results:
        op = r["outP"].reshape(128, NJ, BS).transpose(1, 0, 2).reshape(H, BS)
        outs.append(op)
    out = np.concatenate(outs, axis=1).T
    return np.ascontiguousarray(out).astype(np.float32)



# revision 3
# speedup vs baseline: 1.1029x; 1.1029x over previous
"""CfC (closed-form continuous-time) cell kernel for Trainium2, 8 NeuronCores.

Reference computation (B=8192, IN=256, H=512, all fp32):
    g     = sigmoid(x @ W_gx.T + b_gx + h @ W_gh.T + gate_b)        [B, H]
    f     = tanh(cat([x, h]) @ W_backbone.T + b_backbone)           [B, H]
    tau   = softplus(log_tau) + |g|          (g in (0,1) so |g| == g)
    decay = exp(-delta_t[:, None] * tau)
    out   = decay * h + (1 - decay) * f

Strategy: data-parallel over B (1024 rows per core), weights replicated.
Feature-major on device: activations ship as xh^T [768, B_shard] so the
contraction dim lands on SBUF partitions with no on-device transposes.

Matmul precision split: the GATE matmul runs in fp8-e4m3 DoubleRow perf
mode (2x PE rate; the gate feeds sigmoid -> tau -> exp, so its error is
strongly attenuated), while the BACKBONE matmul (direct path to the output)
stays fp16.  Gate operands are pre-scaled by powers of two (x*8, W*32) to
center them in e4m3's normal range; the 1/256 dequant plus the sigmoid
half-angle 1/2 fold into the ACT scale (1/512).

Epilogue (per output j-tile of 128 features):
    tg    = Tanh(zg/512 + bg/2)        ACT, fp16 out
    f     = Tanh(zf + bb)              ACT, fp16 out
    t     = (tg + 2*softplus'+1) * (-dt/2)   one DVE scalar_tensor_tensor
    decay = Exp(t)                     ACT, fp16
    hmf   = h - f                      GpSimd tensor_tensor (offload)
    p     = decay * hmf                DVE
    o     = p + f                      DVE, fp16 out -> DMA
softplus(log_tau) is computed on the host (it is a [H] constant), so the
scalar engine needs a single ACT table set (tanh+exp) loaded once.
All 16-bit DVE ops hit the 2x packed mode.  Output ships fp16 and is
upcast on the host.  Input DMAs are split into k-pair panels and issued
from three trigger rings (sync HWDGE, scalar HWDGE, vector SWDGE) so the
first matmul's data lands as early as possible.  The last j-tile runs its
backbone matmuls first and a chunk-split epilogue to shorten the tail.
"""

from contextlib import ExitStack

import ml_dtypes
import numpy as np

import concourse.bass as bass
import concourse.mybir as mybir
import concourse.tile as tile
from concourse import bacc
from concourse.bass_utils import run_bass_kernel_spmd

B, IN, H = 8192, 256, 512
NCORES = 8
BS = B // NCORES          # 1024 batch rows per core
KIN = IN + H              # 768 contraction dim
KT = KIN // 128           # 6 k-tiles
NP = KT // 2              # 3 k-pair panels (DoubleRow processes 2 k-tiles)
NJ = H // 128             # 4 partition tiles per output matrix
NCHUNK = 512              # matmul moving free dim per PSUM bank
NCH = BS // NCHUNK        # 2 b-chunks per core

SX = 8.0                  # gate activation pre-scale (power of 2)
SW = 32.0                 # gate weight pre-scale (power of 2)
GDEQ = 1.0 / (SX * SW * 2.0)   # ACT scale: dequant + sigmoid half-angle

F32 = mybir.dt.float32
FP16 = mybir.dt.float16
FP8 = mybir.dt.float8e4
AF = mybir.ActivationFunctionType
OP = mybir.AluOpType
PM = mybir.MatmulPerfMode

TRACE = False             # test.py flips this for profiled runs
LAST_RESULT = None        # BassKernelResults of the most recent run

_NC_CACHE = None


def _body(tc, xg8, w8g, xh16, w16b, ndt2, consts, outP):
    nc = tc.nc
    with ExitStack() as ctx:
        singles = ctx.enter_context(tc.tile_pool(name="singles", bufs=1))
        work = ctx.enter_context(tc.tile_pool(name="work", bufs=3))
        psg = ctx.enter_context(tc.tile_pool(name="psg", bufs=2, space="PSUM"))
        psf = ctx.enter_context(tc.tile_pool(name="psf", bufs=2, space="PSUM"))

        # Persistent SBUF tensors, all partition-major-packed in DRAM so
        # every DMA descriptor row is contiguous.
        xg_sb = singles.tile([128, KT, BS], FP8, tag="xg")
        wg_sb = singles.tile([128, KT, H], FP8, tag="wg")
        xh_sb = singles.tile([128, KT, BS], FP16, tag="xh")
        wb_sb = singles.tile([128, KT, H], FP16, tag="wb")
        ndt_sb = singles.tile([128, BS], FP16, tag="ndt")
        cst = singles.tile([128, 3, NJ], F32, tag="cst")

        xg_v = xg8.rearrange("p (g c) -> p g c", g=NP)
        xh_v = xh16.rearrange("p (g c) -> p g c", g=NP)

        # Gate stream on the sync HWDGE ring: W first (feeds LDWEIGHTS),
        # then xg k-pair panels in consumption order.
        nc.sync.dma_start(out=wg_sb.rearrange("p k n -> p (k n)"), in_=w8g)
        for g in range(NP):
            nc.sync.dma_start(
                out=xg_sb[:, 2 * g:2 * g + 2, :].rearrange("p k b -> p (k b)"),
                in_=xg_v[:, g, :],
            )
        # Backbone stream on the scalar HWDGE ring (scalar is idle until the
        # first ACT at ~7us; its table load slots in after these triggers).
        nc.scalar.dma_start(out=wb_sb.rearrange("p k n -> p (k n)"), in_=w16b)
        for g in range(NP):
            nc.scalar.dma_start(
                out=xh_sb[:, 2 * g:2 * g + 2, :].rearrange("p k b -> p (k b)"),
                in_=xh_v[:, g, :],
            )
        # Small epilogue constants on the gpsimd SWDGE ring (gpsimd's first
        # real op is late).  consts is [bg/2 | bb | 2*softplus(log_tau)+1],
        # each [H]; column j of the SBUF tile is that vector's j-th 128-row
        # slice (per-partition scalars).
        nc.gpsimd.dma_start(out=ndt_sb, in_=ndt2)
        nc.gpsimd.dma_start(
            out=cst, in_=consts.rearrange("(c j p) -> p c j", p=128, j=NJ)
        )

        def gate_mms(j, zg):
            for n in range(NCH):
                bsl = slice(n * NCHUNK, (n + 1) * NCHUNK)
                for g in range(NP):
                    nc.tensor.matmul(
                        zg[:, bsl],
                        wg_sb[:, 2 * g:2 * g + 2, j * 128:(j + 1) * 128],
                        xg_sb[:, 2 * g:2 * g + 2, bsl],
                        start=(g == 0),
                        stop=(g == NP - 1),
                        perf_mode=PM.DoubleRow,
                    )

        def backbone_mms(j, zf):
            for n in range(NCH):
                bsl = slice(n * NCHUNK, (n + 1) * NCHUNK)
                for k in range(KT):
                    nc.tensor.matmul(
                        zf[:, bsl],
                        wb_sb[:, k, j * 128:(j + 1) * 128],
                        xh_sb[:, k, bsl],
                        start=(k == 0),
                        stop=(k == KT - 1),
                    )

        def epilogue(j, zg, zf, csl, name):
            n = csl.stop - csl.start
            tg = work.tile([128, n], FP16, tag="tg", name=f"tg_{name}")
            f = work.tile([128, n], FP16, tag="f", name=f"f_{name}")
            t = work.tile([128, n], FP16, tag="t", name=f"t_{name}")
            dec = work.tile([128, n], FP16, tag="dec", name=f"dec_{name}")
            hmf = work.tile([128, n], FP16, tag="hmf", name=f"hmf_{name}")
            p = work.tile([128, n], FP16, tag="p", name=f"p_{name}")
            o = work.tile([128, n], FP16, tag="o", name=f"o_{name}")

            # tg = tanh((zg + 256*bg)/512) = tanh(zg_true/2 + bg/2)
            nc.scalar.activation(
                out=tg, in_=zg[:, csl], func=AF.Tanh, bias=cst[:, 0, j:j + 1],
                scale=GDEQ,
            )
            nc.scalar.activation(
                out=f, in_=zf[:, csl], func=AF.Tanh, bias=cst[:, 1, j:j + 1]
            )
            # t = (tg + (2*softplus+1)) * (-dt/2)  [= -dt * (softplus + g)]
            nc.vector.scalar_tensor_tensor(
                out=t, in0=tg, scalar=cst[:, 2, j:j + 1], in1=ndt_sb[:, csl],
                op0=OP.add, op1=OP.mult,
            )
            nc.scalar.activation(out=dec, in_=t, func=AF.Exp)
            # h rows live in xh_sb k-tiles 2..5
            nc.gpsimd.tensor_sub(out=hmf, in0=xh_sb[:, 2 + j, csl], in1=f)
            nc.vector.tensor_mul(out=p, in0=dec, in1=hmf)
            nc.vector.tensor_add(out=o, in0=p, in1=f)
            nc.sync.dma_start(
                out=outP[:, j * BS + csl.start:j * BS + csl.stop], in_=o
            )

        for j in range(NJ):
            zg = psg.tile([128, BS], F32, tag="zg", name=f"zg_{j}")
            zf = psf.tile([128, BS], F32, tag="zf", name=f"zf_{j}")
            if j < NJ - 1:
                # Gate first: its (shorter) matmul block finishes early so
                # the scalar engine starts while backbone matmuls still run.
                gate_mms(j, zg)
                backbone_mms(j, zf)
                epilogue(j, zg, zf, slice(0, BS), f"{j}")
            else:
                # Last tile: backbone first + chunk-split epilogue to
                # shorten the post-matmul tail.
                backbone_mms(j, zf)
                gate_mms(j, zg)
                for c in range(NCH):
                    epilogue(j, zg, zf,
                             slice(c * NCHUNK, (c + 1) * NCHUNK), f"{j}_{c}")


def build_nc():
    nc = bacc.Bacc(
        "TRN2",
        target_bir_lowering=False,
        debug=False,
        enable_asserts=False,
        num_devices=NCORES,
    )
    # Partition-major packed streams: row p holds that partition's entire
    # contiguous payload.
    xg8 = nc.dram_tensor("xg8", [128, KT * BS], FP8, kind="ExternalInput").ap()
    w8g = nc.dram_tensor("w8g", [128, KT * H], FP8, kind="ExternalInput").ap()
    xh16 = nc.dram_tensor("xh16", [128, KT * BS], FP16, kind="ExternalInput").ap()
    w16b = nc.dram_tensor("w16b", [128, KT * H], FP16, kind="ExternalInput").ap()
    ndt2 = nc.dram_tensor("ndt2", [128, BS], FP16, kind="ExternalInput").ap()
    consts = nc.dram_tensor("consts", [3 * H], F32, kind="ExternalInput").ap()
    outP = nc.dram_tensor("outP", [128, NJ * BS], FP16, kind="ExternalOutput").ap()
    with tile.TileContext(nc) as tc:
        _body(tc, xg8, w8g, xh16, w16b, ndt2, consts, outP)
    nc.compile()
    return nc


def _get_nc():
    global _NC_CACHE
    if _NC_CACHE is None:
        _NC_CACHE = build_nc()
    return _NC_CACHE


def _pack_pmajor(a, kt):
    """[kt*128, C] -> [128, kt*C]: partition-major pack so each of the 128
    DMA rows is contiguous in DRAM."""
    c = a.shape[1]
    return np.ascontiguousarray(
        a.reshape(kt, 128, c).transpose(1, 0, 2).reshape(128, kt * c)
    )


def make_in_maps(x, h, delta_t, W_backbone, b_backbone, W_gx, b_gx, W_gh,
                 gate_b, log_tau):
    f32 = np.float32
    xh = np.concatenate(
        [np.asarray(x, f32), np.asarray(h, f32)], axis=1
    )                                                   # [B, 768]
    xhT = np.ascontiguousarray(xh.T)                    # [768, B] f32
    xh16 = xhT.astype(np.float16)
    xg8 = np.asarray(xhT * SX, dtype=ml_dtypes.float8_e4m3)

    WgT = np.concatenate(
        [np.asarray(W_gx, f32), np.asarray(W_gh, f32)], axis=1
    ).T                                                 # [768, H]
    w8g = _pack_pmajor(np.asarray(WgT * SW, dtype=ml_dtypes.float8_e4m3), KT)
    w16b = _pack_pmajor(
        np.ascontiguousarray(np.asarray(W_backbone, f32).T).astype(np.float16),
        KT,
    )

    sp2 = 2.0 * np.log1p(np.exp(np.asarray(log_tau, f32))) + 1.0
    consts = np.concatenate(
        [
            (np.asarray(b_gx, f32) + np.asarray(gate_b, f32)) * 0.5,
            np.asarray(b_backbone, f32),
            sp2,
        ]
    ).astype(f32)                                       # [3H]
    ndt2 = (np.asarray(delta_t, f32) * -0.5).astype(np.float16)   # [B]

    in_maps = []
    for c in range(NCORES):
        sl = slice(c * BS, (c + 1) * BS)
        in_maps.append(
            {
                "xg8": _pack_pmajor(xg8[:, sl], KT),
                "w8g": w8g,
                "xh16": _pack_pmajor(xh16[:, sl], KT),
                "w16b": w16b,
                "ndt2": np.ascontiguousarray(
                    np.broadcast_to(ndt2[sl][None, :], (128, BS))
                ),
                "consts": consts,
            }
        )
    return in_maps


def kernel(x, h, delta_t, W_backbone, b_backbone, W_gx, b_gx, W_gh, gate_b,
           log_tau):
    global LAST_RESULT
    in_maps = make_in_maps(x, h, delta_t, W_backbone, b_backbone, W_gx, b_gx,
                           W_gh, gate_b, log_tau)
    nc = _get_nc()
    res = run_bass_kernel_spmd(
        nc, in_maps, core_ids=list(range(NCORES)), trace=TRACE
    )
    LAST_RESULT = res
    # outP is [128, NJ*BS] partition-major; unpack to [H, BS] then gather.
    outs = []
    for r in res.results:
        op = r["outP"].reshape(128, NJ, BS).transpose(1, 0, 2).reshape(H, BS)
        outs.append(op)
    out = np.concatenate(outs, axis=1).T
    return np.ascontiguousarray(out).astype(np.float32)


# revision 6
# speedup vs baseline: 1.2753x; 1.1564x over previous
"""CfC (closed-form continuous-time) cell kernel for Trainium2, 8 NeuronCores.

Reference computation (B=8192, IN=256, H=512, all fp32):
    g     = sigmoid(x @ W_gx.T + b_gx + h @ W_gh.T + gate_b)        [B, H]
    f     = tanh(cat([x, h]) @ W_backbone.T + b_backbone)           [B, H]
    tau   = softplus(log_tau) + |g|          (g in (0,1) so |g| == g)
    decay = exp(-delta_t[:, None] * tau)
    out   = decay * h + (1 - decay) * f

Strategy: data-parallel over B (1024 rows per core), weights replicated.
Feature-major on device: activations ship as xh^T [768, B_shard] so the
contraction dim lands on SBUF partitions with no on-device transposes.

Matmul precision split: the GATE matmul runs in fp8-e4m3 DoubleRow perf
mode (2x PE rate; the gate feeds sigmoid -> tau -> exp, so its error is
strongly attenuated), while the BACKBONE matmul (direct path to the output)
stays fp16.  Gate operands are pre-scaled by powers of two (x*8, W*32) to
center them in e4m3's normal range; the 1/256 dequant plus the sigmoid
half-angle 1/2 fold into the ACT scale (1/512).

Two-phase schedule, sized by the input-DMA roofline (~3.7MB at ~300GB/s):
the gate phase needs only the small fp8 streams (1.15MB) so it starts as
soon as they land; its epilogue chain (tg -> t -> decay) has no backbone
dependency.  The backbone phase (fp16, 2.3MB) streams in behind it.  Both
phases emit matmuls j-tile by j-tile through a single 4-deep PSUM ring.

    phase A, per j:  zg = DoubleRow-fp8 matmuls
                     tg    = Tanh(zg/512 + bg/2)              ACT
                     t     = (tg + 2*softplus'+1) * (-dt/2)   DVE STT
                     decay = Exp(t)                           ACT
    phase B, per j:  zf = fp16 matmuls
                     f     = Tanh(zf + bb)                    ACT
                     hmf   = h - f                            DVE
                     p     = decay * hmf                      DVE
                     o     = p + f -> DMA out (fp16)          DVE

softplus(log_tau) is a [H] constant computed on the host, so the scalar
engine needs one ACT table set (tanh+exp) loaded once.  All 16-bit DVE ops
hit the 2x packed mode.  Activations are packed chunk-major so each DMA is
one contiguous-row panel in matmul consumption order; triggers are split
across the two HWDGE rings (sync: gate stream, scalar: backbone stream)
plus gpsimd SWDGE for -dt/2.  The last j-tile runs a chunk-split epilogue
to shorten the post-matmul tail.  Output ships fp16, upcast on the host.
"""

from contextlib import ExitStack

import ml_dtypes
import numpy as np

import concourse.bass as bass
import concourse.mybir as mybir
import concourse.tile as tile
from concourse import bacc
from concourse.bass_utils import run_bass_kernel_spmd

B, IN, H = 8192, 256, 512
NCORES = 8
BS = B // NCORES          # 1024 batch rows per core
KIN = IN + H              # 768 contraction dim
KT = KIN // 128           # 6 k-tiles
NP = KT // 2              # 3 k-pair panels (DoubleRow processes 2 k-tiles)
NJ = H // 128             # 4 partition tiles per output matrix
NCHUNK = 512              # matmul moving free dim per PSUM bank
NCH = BS // NCHUNK        # 2 b-chunks per core

SX = 8.0                  # gate activation pre-scale (power of 2)
SW = 32.0                 # gate weight pre-scale (power of 2)
GDEQ = 1.0 / (SX * SW * 2.0)   # ACT scale: dequant + sigmoid half-angle

F32 = mybir.dt.float32
FP16 = mybir.dt.float16
FP8 = mybir.dt.float8e4
AF = mybir.ActivationFunctionType
OP = mybir.AluOpType
PM = mybir.MatmulPerfMode

TRACE = False             # test.py flips this for profiled runs
LAST_RESULT = None        # BassKernelResults of the most recent run

_NC_CACHE = None


def _body(tc, xg8, w8g, xh16, w16b, ndt2, consts, outP):
    nc = tc.nc
    with ExitStack() as ctx:
        singles = ctx.enter_context(tc.tile_pool(name="singles", bufs=1))
        decs = ctx.enter_context(tc.tile_pool(name="decs", bufs=NJ))
        work = ctx.enter_context(tc.tile_pool(name="work", bufs=2))
        psz = ctx.enter_context(tc.tile_pool(name="psz", bufs=4, space="PSUM"))

        # Persistent SBUF tensors.  Activation streams are chunk-major
        # ([128, chunk, k, 512]) so each chunk panel is one contiguous-row
        # DMA in matmul consumption order.
        xg_sb = singles.tile([128, NCH, KT, NCHUNK], FP8, tag="xg")
        wg_sb = singles.tile([128, KT, H], FP8, tag="wg")
        xh_sb = singles.tile([128, NCH, KT, NCHUNK], FP16, tag="xh")
        wb_sb = singles.tile([128, KT, H], FP16, tag="wb")
        ndt_sb = singles.tile([128, BS], FP16, tag="ndt")
        cst = singles.tile([128, 3, NJ], F32, tag="cst")

        CC = KT * NCHUNK
        # Gate-critical stream on the sync HWDGE ring.  cst is tiny (6KB)
        # and feeds the first ACT's bias slot, so it goes first.
        nc.sync.dma_start(out=cst.rearrange("p c j -> p (c j)"),
                          in_=consts)
        nc.sync.dma_start(out=wg_sb.rearrange("p k n -> p (k n)"), in_=w8g)
        for n in range(NCH):
            nc.sync.dma_start(
                out=xg_sb[:, n].rearrange("p k c -> p (k c)"),
                in_=xg8[:, n * CC:(n + 1) * CC],
            )
        # Backbone stream on the scalar HWDGE ring (scalar is idle until its
        # first ACT; the table load slots in around these triggers).
        nc.scalar.dma_start(out=wb_sb.rearrange("p k n -> p (k n)"), in_=w16b)
        for n in range(NCH):
            nc.scalar.dma_start(
                out=xh_sb[:, n].rearrange("p k c -> p (k c)"),
                in_=xh16[:, n * CC:(n + 1) * CC],
            )
        # -dt/2 broadcast rides the gpsimd SWDGE ring.
        nc.gpsimd.dma_start(out=ndt_sb, in_=ndt2)

        # --- Phase A: gate matmuls + gate chain (tg -> t -> decay) ---
        dec = {}
        for j in range(NJ):
            zg = psz.tile([128, BS], F32, tag="z", name=f"zg_{j}")
            for n in range(NCH):
                bsl = slice(n * NCHUNK, (n + 1) * NCHUNK)
                for g in range(NP):
                    nc.tensor.matmul(
                        zg[:, bsl],
                        wg_sb[:, 2 * g:2 * g + 2, j * 128:(j + 1) * 128],
                        xg_sb[:, n, 2 * g:2 * g + 2, :],
                        start=(g == 0),
                        stop=(g == NP - 1),
                        perf_mode=PM.DoubleRow,
                    )
            tg = work.tile([128, BS], FP16, tag="tg", name=f"tg_{j}")
            t = work.tile([128, BS], FP16, tag="t", name=f"t_{j}")
            dec[j] = decs.tile([128, BS], FP16, tag="dec", name=f"dec_{j}")
            # tg = tanh((zg + 256*bg)/512) = tanh(zg_true/2 + bg/2)
            nc.scalar.activation(
                out=tg, in_=zg, func=AF.Tanh, bias=cst[:, 0, j:j + 1],
                scale=GDEQ,
            )
            # t = (tg + (2*softplus+1)) * (-dt/2)  [= -dt * (softplus + g)]
            nc.vector.scalar_tensor_tensor(
                out=t, in0=tg, scalar=cst[:, 2, j:j + 1], in1=ndt_sb,
                op0=OP.add, op1=OP.mult,
            )
            nc.scalar.activation(out=dec[j], in_=t, func=AF.Exp)

        # --- Phase B: backbone matmuls + f + combine ---
        def combine(j, zf, csl, name, sfx):
            n = csl.stop - csl.start
            cs = slice(csl.start // NCHUNK, max(1, csl.stop // NCHUNK))
            f = work.tile([128, n], FP16, tag=f"f{sfx}", name=f"f_{name}")
            hmf = work.tile([128, n], FP16, tag=f"hmf{sfx}", name=f"hmf_{name}")
            p = work.tile([128, n], FP16, tag=f"p{sfx}", name=f"p_{name}")
            o = work.tile([128, n], FP16, tag=f"o{sfx}", name=f"o_{name}")
            nc.scalar.activation(
                out=f, in_=zf[:, csl], func=AF.Tanh, bias=cst[:, 1, j:j + 1]
            )
            # h rows live in xh_sb k-tiles 2..5 (chunk-major view)
            nc.vector.tensor_sub(
                out=hmf.rearrange("p (c x) -> p c x", x=NCHUNK),
                in0=xh_sb[:, cs, 2 + j, :],
                in1=f.rearrange("p (c x) -> p c x", x=NCHUNK),
            )
            nc.vector.tensor_mul(out=p, in0=dec[j][:, csl], in1=hmf)
            nc.vector.tensor_add(out=o, in0=p, in1=f)
            nc.sync.dma_start(
                out=outP[:, j * BS + csl.start:j * BS + csl.stop], in_=o
            )

        for j in range(NJ):
            zf = psz.tile([128, BS], F32, tag="z", name=f"zf_{j}")
            for n in range(NCH):
                bsl = slice(n * NCHUNK, (n + 1) * NCHUNK)
                for k in range(KT):
                    nc.tensor.matmul(
                        zf[:, bsl],
                        wb_sb[:, k, j * 128:(j + 1) * 128],
                        xh_sb[:, n, k, :],
                        start=(k == 0),
                        stop=(k == KT - 1),
                    )
            if j < NJ - 1:
                combine(j, zf, slice(0, BS), f"{j}", "")
            else:
                # Chunk-split tail so the last DMA leaves sooner.
                for c in range(NCH):
                    combine(j, zf, slice(c * NCHUNK, (c + 1) * NCHUNK),
                            f"{j}_{c}", "c")


def build_nc():
    nc = bacc.Bacc(
        "TRN2",
        target_bir_lowering=False,
        debug=False,
        enable_asserts=False,
        num_devices=NCORES,
    )
    # Partition-major packed streams: row p holds that partition's entire
    # contiguous payload.
    xg8 = nc.dram_tensor("xg8", [128, KT * BS], FP8, kind="ExternalInput").ap()
    w8g = nc.dram_tensor("w8g", [128, KT * H], FP8, kind="ExternalInput").ap()
    xh16 = nc.dram_tensor("xh16", [128, KT * BS], FP16, kind="ExternalInput").ap()
    w16b = nc.dram_tensor("w16b", [128, KT * H], FP16, kind="ExternalInput").ap()
    ndt2 = nc.dram_tensor("ndt2", [128, BS], FP16, kind="ExternalInput").ap()
    consts = nc.dram_tensor("consts", [128, 3 * NJ], F32,
                            kind="ExternalInput").ap()
    outP = nc.dram_tensor("outP", [128, NJ * BS], FP16, kind="ExternalOutput").ap()
    with tile.TileContext(nc) as tc:
        _body(tc, xg8, w8g, xh16, w16b, ndt2, consts, outP)
    nc.compile()
    return nc


def _get_nc():
    global _NC_CACHE
    if _NC_CACHE is None:
        _NC_CACHE = build_nc()
    return _NC_CACHE


def _pack_cmajor(a, kt, nch, nchunk):
    """[kt*128, nch*nchunk] -> [128, nch*kt*nchunk] chunk-major pack: row p
    holds [chunk0: k0..k5 | chunk1: k0..k5], each 128-partition-sliced."""
    return np.ascontiguousarray(
        a.reshape(kt, 128, nch, nchunk).transpose(1, 2, 0, 3)
        .reshape(128, nch * kt * nchunk)
    )


def _pack_pmajor(a, kt):
    """[kt*128, C] -> [128, kt*C]: partition-major pack so each of the 128
    DMA rows is contiguous in DRAM."""
    c = a.shape[1]
    return np.ascontiguousarray(
        a.reshape(kt, 128, c).transpose(1, 0, 2).reshape(128, kt * c)
    )


def make_in_maps(x, h, delta_t, W_backbone, b_backbone, W_gx, b_gx, W_gh,
                 gate_b, log_tau):
    f32 = np.float32
    xh = np.concatenate(
        [np.asarray(x, f32), np.asarray(h, f32)], axis=1
    )                                                   # [B, 768]
    xhT = np.ascontiguousarray(xh.T)                    # [768, B] f32
    xh16 = xhT.astype(np.float16)
    xg8 = np.asarray(xhT * SX, dtype=ml_dtypes.float8_e4m3)

    WgT = np.concatenate(
        [np.asarray(W_gx, f32), np.asarray(W_gh, f32)], axis=1
    ).T                                                 # [768, H]
    w8g = _pack_pmajor(np.asarray(WgT * SW, dtype=ml_dtypes.float8_e4m3), KT)
    w16b = _pack_pmajor(
        np.ascontiguousarray(np.asarray(W_backbone, f32).T).astype(np.float16),
        KT,
    )

    sp2 = 2.0 * np.log1p(np.exp(np.asarray(log_tau, f32))) + 1.0
    # cstP[p, c*NJ+j] = const_c[j*128+p]
    cstv = np.stack(
        [
            (np.asarray(b_gx, f32) + np.asarray(gate_b, f32)) * 0.5,
            np.asarray(b_backbone, f32),
            sp2,
        ]
    )                                                   # [3, H]
    cstP = np.ascontiguousarray(
        cstv.reshape(3, NJ, 128).transpose(2, 0, 1).reshape(128, 3 * NJ)
    ).astype(f32)
    ndt2 = (np.asarray(delta_t, f32) * -0.5).astype(np.float16)   # [B]

    in_maps = []
    for c in range(NCORES):
        sl = slice(c * BS, (c + 1) * BS)
        in_maps.append(
            {
                "xg8": _pack_cmajor(xg8[:, sl], KT, NCH, NCHUNK),
                "w8g": w8g,
                "xh16": _pack_cmajor(xh16[:, sl], KT, NCH, NCHUNK),
                "w16b": w16b,
                "ndt2": np.ascontiguousarray(
                    np.broadcast_to(ndt2[sl][None, :], (128, BS))
                ),
                "consts": cstP,
            }
        )
    return in_maps


def kernel(x, h, delta_t, W_backbone, b_backbone, W_gx, b_gx, W_gh, gate_b,
           log_tau):
    global LAST_RESULT
    in_maps = make_in_maps(x, h, delta_t, W_backbone, b_backbone, W_gx, b_gx,
                           W_gh, gate_b, log_tau)
    nc = _get_nc()
    res = run_bass_kernel_spmd(
        nc, in_maps, core_ids=list(range(NCORES)), trace=TRACE
    )
    LAST_RESULT = res
    # outP is [128, NJ*BS] partition-major; unpack to [H, BS] then gather.
    outs = []
    for r in res.results:
        op = r["outP"].reshape(128, NJ, BS).transpose(1, 0, 2).reshape(H, BS)
        outs.append(op)
    out = np.concatenate(outs, axis=1).T
    return np.ascontiguousarray(out).astype(np.float32)
